# revision 23
# baseline (speedup 1.0000x reference)
"""Causal GQA self-attention (B=2, T=2048, C=1024, 16 q-heads / 4 kv-heads,
RoPE, causal softmax, output projection) on 8 Trainium2 NeuronCores.

Sharding: core c = b*4 + hg handles batch b (2-way data parallel) and
head-group hg (4-way tensor parallel: its 4 q-heads + their shared kv head).
W_qkv is column-sharded, W_proj row-sharded; each core emits a partial
projection [2048, 1024] and the host sums the 4 partials per batch.

Device pipeline per core (bf16 matmul inputs, fp32 PSUM accumulate):
  1. q^T and k^T computed DIRECTLY in [head-dim, token] layout:
     qT_raw = W_stack^T @ x^T (host pre-transposes; two 64-dim q heads per
     128-wide block; the k block is [W_k | W_k] so k^T lands duplicated on
     both partition halves for the two-head score trick). RoPE pair-swap
     comes from ONE cheap 128x128 permutation matmul (pswap) per stack:
     qT_rot = qT_raw*cos + (P @ qT_raw)*ssin, with the rotation sign baked
     into the host ssin table. No DMA transposes for q/k.
  2. v^T = W_v^T @ x^T (8 wide matmuls), then 4 small DMA-xbar transposes
     per chunk into natural [token, dim] layout + ones column (softmax
     denominator row via the [1|v] stationary trick).
  3. flash-style per 512-wide q chunk: for each 128-token k tile, the two
     heads' scores land in ONE 2-bank PSUM tile [128, 2, 512]; a single exp
     on ScalarE covers both heads (1/sqrt(64) folded into the activation
     scale); causal masking via one 3D affine_select on diagonal blocks;
     y^T[65, 2, q] += [1|v]^T @ P^T with a single merged matmul.
  4. y^T row 0 is the softmax denominator: reciprocal_approx + GPSIMD
     partition_broadcast + one fused scale-evacuate multiply
  5. out = yT.T @ W_proj_shard, stored partition-major with one DMA per
     chunk (128 descriptors each).
"""

import sys

if "/opt/trn_rl_repo" not in sys.path:
    sys.path.insert(0, "/opt/trn_rl_repo")

import numpy as np
import ml_dtypes

B, T, C = 2, 2048, 1024
NH, NKV, HD = 16, 4, 64
THETA = 10000.0
NQ = NH // NKV          # q heads per core = 4
TT = T // 128           # 16 token tiles
NCHUNK = T // 512       # 4 q-chunks
BF16 = ml_dtypes.bfloat16

_CACHE = {}


def _null_ctx():
    from contextlib import nullcontext

    return nullcontext()


def _build(reps=1, merged_pv=False):
    """Build the SPMD Bass program (identical on all 8 cores).

    reps>1 wraps the whole body in a hardware loop (constant NEFF size) —
    used only by hw_time.py to measure per-iteration device time.
    """
    import concourse.mybir as mybir
    import concourse.tile as tile
    from concourse import bacc
    from concourse.bass import ts
    from contextlib import ExitStack

    dt = mybir.dt
    AF = mybir.ActivationFunctionType

    nc = bacc.Bacc("TRN2", target_bir_lowering=False, debug=False, num_devices=8)

    # host pre-shuffled, partition-major inputs (contiguous per partition)
    xt_d = nc.declare_dram_parameter("xT", [128, 8 * T], dt.bfloat16, isOutput=False)
    wq_d = nc.declare_dram_parameter("wqT", [128, 8 * 384], dt.bfloat16, isOutput=False)
    wv_d = nc.declare_dram_parameter("wv", [128, 8 * 64], dt.bfloat16, isOutput=False)
    ps_d = nc.declare_dram_parameter("pswap", [128, 128], dt.bfloat16, isOutput=False)
    cs_d = nc.declare_dram_parameter("cs2", [128, 2 * T], dt.bfloat16, isOutput=False)
    wo_d = nc.declare_dram_parameter("wo", [128, 2 * C], dt.bfloat16, isOutput=False)
    # partition-major output: row p holds (p, tt, :) — 128 descriptors/DMA
    out_d = nc.declare_dram_parameter("out", [128, TT * C], dt.bfloat16, isOutput=True)

    with tile.TileContext(nc) as tc:
     with (tc.For_i(0, reps) if reps > 1 else _null_ctx()):
      with ExitStack() as ctx:
        persist = ctx.enter_context(tc.tile_pool(name="persist", bufs=1))
        cmb_tmp = ctx.enter_context(tc.tile_pool(name="cmb_tmp", bufs=3))
        raw_pool = ctx.enter_context(tc.tile_pool(name="raw", bufs=3))
        p_pool = ctx.enter_context(tc.tile_pool(name="p_pool", bufs=8))
        po_pool = ctx.enter_context(tc.tile_pool(name="po", bufs=2))
        ysb_pool = ctx.enter_context(tc.tile_pool(name="ysb", bufs=3))
        bc_pool = ctx.enter_context(tc.tile_pool(name="bc", bufs=2))
        yst_pool = ctx.enter_context(tc.tile_pool(name="yst", bufs=6))
        s_ps_pool = ctx.enter_context(
            tc.tile_pool(name="s_ps", bufs=2, space="PSUM")
        )
        qk_ps_pool = ctx.enter_context(
            tc.tile_pool(name="qk_ps", bufs=2, space="PSUM")
        )
        y_ps_pool = ctx.enter_context(
            tc.tile_pool(name="y_ps", bufs=1, space="PSUM")
        )

        # ---- persistent SBUF; DMA order tuned so the first qkv matmul
        # group (xt chunk 0 + wq + pswap) lands before cs/wv/wo ----
        wq_sb = persist.tile([128, 8, 384], dt.bfloat16)
        ps_sb = persist.tile([128, 128], dt.bfloat16)
        wv_sb = persist.tile([128, 8, 64], dt.bfloat16)
        cs_sb = persist.tile([128, 2, T], dt.bfloat16)
        wo_sb = persist.tile([128, 2, C], dt.bfloat16)
        xt_sb = [
            persist.tile([128, 8, 512], dt.bfloat16, name=f"xtc{jc}")
            for jc in range(NCHUNK)
        ]
        nc.sync.dma_start(
            wq_sb[:, 0:2, :],
            wq_d.ap()[:, 0 : 2 * 384].rearrange("p (c n) -> p c n", c=2),
        )
        nc.sync.dma_start(ps_sb[:], ps_d.ap())
        nc.sync.dma_start(
            xt_sb[0][:, 0:2, :],
            xt_d.ap()[:, 0:1024].rearrange("p (c t) -> p c t", c=2),
        )
        nc.sync.dma_start(
            wq_sb[:, 2:8, :],
            wq_d.ap()[:, 2 * 384 :].rearrange("p (c n) -> p c n", c=6),
        )
        nc.sync.dma_start(
            xt_sb[0][:, 2:8, :],
            xt_d.ap()[:, 1024 : 8 * 512].rearrange("p (c t) -> p c t", c=6),
        )
        nc.sync.dma_start(
            cs_sb[:], cs_d.ap().rearrange("p (v t) -> p v t", v=2)
        )
        nc.sync.dma_start(
            wv_sb[:], wv_d.ap().rearrange("p (c n) -> p c n", c=8)
        )
        for jc in range(1, NCHUNK):
            nc.sync.dma_start(
                xt_sb[jc][:],
                xt_d.ap()[:, ts(jc, 8 * 512)].rearrange("p (c t) -> p c t", c=8),
            )
        nc.sync.dma_start(
            wo_sb[:], wo_d.ap().rearrange("p (c n) -> p c n", c=2)
        )

        qt_sb = [[None] * NCHUNK for _ in range(2)]   # [hp][chunk] [128,512]
        kt_sb = []                                    # per chunk [128,512] (dup)
        va_sb = []                                    # per chunk [128,4,65]
        ynt = [[None] * NCHUNK for _ in range(2)]     # [dimtile][chunk] [128,512]
        for d in range(2):
            for j in range(NCHUNK):
                qt_sb[d][j] = persist.tile([128, 512], dt.bfloat16, name=f"qt{d}_{j}")
                ynt[d][j] = persist.tile([128, 512], dt.bfloat16, name=f"ynt{d}_{j}")

        # ---- phase 1: qT/kT via matmul + perm-matmul rope; vT + transpose ----
        for jc in range(NCHUNK):
            kt = persist.tile([128, 512], dt.bfloat16, name=f"kt{jc}")
            kt_sb.append(kt)
            va = persist.tile([128, 4, 65], dt.bfloat16, name=f"va{jc}")
            va_sb.append(va)
            cos_sl = cs_sb[:, 0, ts(jc, 512)]
            sin_sl = cs_sb[:, 1, ts(jc, 512)]
            # block order in wqT: q0 | q1 | k
            for bA, dst in ((2, kt), (0, qt_sb[0][jc]), (1, qt_sb[1][jc])):
                psA = qk_ps_pool.tile([128, 512], dt.float32, tag="qk", name="psA")
                for c in range(8):
                    nc.tensor.matmul(
                        psA[:],
                        lhsT=wq_sb[:, c, ts(bA, 128)],
                        rhs=xt_sb[jc][:, c, :],
                        start=(c == 0),
                        stop=(c == 7),
                    )
                raw = raw_pool.tile([128, 512], dt.bfloat16, tag="raw")
                nc.vector.tensor_copy(raw[:], psA[:])
                psB = qk_ps_pool.tile([128, 512], dt.float32, tag="qk", name="psB")
                nc.tensor.matmul(
                    psB[:], lhsT=ps_sb[:], rhs=raw[:], start=True, stop=True
                )
                t1 = cmb_tmp.tile([128, 512], dt.bfloat16, tag="t1")
                t2 = cmb_tmp.tile([128, 512], dt.bfloat16, tag="t2")
                nc.vector.tensor_mul(t1[:], raw[:], cos_sl)
                nc.vector.tensor_mul(t2[:], psB[:], sin_sl)
                nc.vector.tensor_add(dst[:], t1[:], t2[:])
            # vT then 4 xbar transposes into natural layout (off crit path)
            psV = qk_ps_pool.tile([64, 512], dt.float32, tag="qk", name="psV")
            for c in range(8):
                nc.tensor.matmul(
                    psV[:],
                    lhsT=wv_sb[:, c, :],
                    rhs=xt_sb[jc][:, c, :],
                    start=(c == 0),
                    stop=(c == 7),
                )
            vt = raw_pool.tile([64, 512], dt.bfloat16, tag="vt")
            nc.vector.tensor_copy(vt[:], psV[:])
            vn = raw_pool.tile([128, 4, 64], dt.bfloat16, tag="vn")
            for t4 in range(4):
                nc.sync.dma_start_transpose(
                    vn[:, t4, :], vt[:, ts(t4, 128)]
                )
            nc.vector.tensor_copy(va[:, :, 1:65], vn[:])
            nc.gpsimd.memset(va[:, :, 0:1], 1.0)

        # ---- phase 3+4: attention + projection per 512-wide q chunk ----
        # y^T[65, 2, q] = [1|v]^T @ P^T over k tiles; row 0 = denominator.
        # Both heads of a pair share one 2-bank score tile and one exp.
        for j in range(NCHUNK):
            for hp in range(2):
                y_ps = y_ps_pool.tile(
                    [65, 2, 512], dt.float32, tag="y", name="y_ps"
                )
                for i in range(4 * j + 4):  # k tiles
                    ic, i4 = divmod(i, 4)
                    off = max(0, 128 * i - 512 * j)  # causal: valid q >= 128*i
                    s2 = s_ps_pool.tile(
                        [128, 2, 512], dt.float32, tag="s", name="s2"
                    )
                    for u in range(2):  # head 2hp+u, kT copy at partitions 64u
                        nc.tensor.matmul(
                            s2[:, u, off:512],
                            lhsT=kt_sb[ic][ts(u, 64), ts(i4, 128)],
                            rhs=qt_sb[hp][j][ts(u, 64), off:512],
                            start=True,
                            stop=True,
                        )
                    p_t = p_pool.tile([128, 2, 512], dt.bfloat16, name="p_t")
                    nc.scalar.activation(
                        p_t[:, :, off:512], s2[:, :, off:512], AF.Exp, scale=0.125
                    )
                    if 128 * i >= 512 * j:  # diagonal block: causal mask
                        # keep where q_local - k_local >= 0, else 0 (both heads)
                        nc.gpsimd.affine_select(
                            p_t[:, :, off : off + 128],
                            p_t[:, :, off : off + 128],
                            pattern=[[0, 2], [1, 128]],
                            compare_op=mybir.AluOpType.is_ge,
                            fill=0.0,
                            base=0,
                            channel_multiplier=-1,
                        )
                    if merged_pv:
                        nc.tensor.matmul(
                            y_ps[:, :, off:512],
                            lhsT=va_sb[ic][:, i4, 0:65],
                            rhs=p_t[:, :, off:512],
                            start=(i == 0),
                            stop=(i == 4 * j + 3),
                        )
                    else:
                        for u in range(2):
                            nc.tensor.matmul(
                                y_ps[:, u, off:512],
                                lhsT=va_sb[ic][:, i4, 0:65],
                                rhs=p_t[:, u, off:512],
                                start=(i == 0),
                                stop=(i == 4 * j + 3),
                            )
                # reciprocal straight off the tiny PSUM den row FIRST so the
                # Pool broadcast overlaps the DVE y evacuation
                den = ysb_pool.tile([1, 2, 512], dt.float32, tag="den")
                nc.vector.reciprocal_approx_fast(den[:], y_ps[0:1, :, :])
                bc = bc_pool.tile([65, 2, 512], dt.float32)
                nc.gpsimd.partition_broadcast(bc[:], den[:], channels=65)
                y_sb = ysb_pool.tile([65, 2, 512], dt.float32)
                nc.vector.tensor_copy(y_sb[:], y_ps[:])
                for u in range(2):
                    yst = yst_pool.tile([65, 512], dt.bfloat16)
                    nc.vector.tensor_mul(yst[:, :], y_sb[:, u, :], bc[:, u, :])
                    if j == NCHUNK - 1 and hp == 1:
                        nc.scalar.dma_start(
                            ynt[hp][j][ts(u, 64), :], yst[1:65, :]
                        )
                    else:
                        nc.gpsimd.dma_start(
                            ynt[hp][j][ts(u, 64), :], yst[1:65, :]
                        )
            # projection for this chunk's 4 token tiles, one store per chunk
            po = po_pool.tile([128, 4, C], dt.bfloat16)
            for t4 in range(4):
                for nn2 in range(2):
                    ps = qk_ps_pool.tile(
                        [128, 512], dt.float32, tag="qk", name="pr_ps"
                    )
                    for dtile in range(2):
                        nc.tensor.matmul(
                            ps[:],
                            lhsT=ynt[dtile][j][:, ts(t4, 128)],
                            rhs=wo_sb[:, dtile, ts(nn2, 512)],
                            start=(dtile == 0),
                            stop=(dtile == 1),
                        )
                    if nn2 == 0:
                        nc.scalar.copy(po[:, t4, ts(nn2, 512)], ps[:])
                    else:
                        nc.vector.tensor_copy(po[:, t4, ts(nn2, 512)], ps[:])
            nc.sync.dma_start(
                out_d.ap()[:, ts(j, 4 * C)],
                po[:].rearrange("p t n -> p (t n)"),
            )

    nc.finalize()
    return nc


def _host_inputs(x, W_qkv, W_proj):
    """Per-core input maps (host-side sharding + partition-major layout)."""
    perm = np.concatenate([np.arange(0, HD, 2), np.arange(1, HD, 2)])  # even|odd
    inv = 1.0 / THETA ** (np.arange(0, HD, 2, dtype=np.float64) / HD)  # [32]
    ang = np.arange(T, dtype=np.float64)[:, None] * inv[None, :]       # [T, 32]
    # cos rows p: cos(ang[t, p mod 32]); ssin rows: -sin on the x1 half
    # (p mod 64 < 32), +sin on the x2 half
    cosT = np.cos(ang).T                                               # [32, T]
    sinT = np.sin(ang).T
    cos128 = np.tile(cosT, (4, 1))                                     # [128, T]
    ssin64 = np.concatenate([-sinT, sinT], axis=0)                     # [64, T]
    ssin128 = np.tile(ssin64, (2, 1))
    cs2 = np.concatenate([cos128, ssin128], axis=1).astype(BF16)       # [128,2T]
    # pair-swap permutation: pswap[k, r] = 1 iff r == k xor 32
    r_idx = np.arange(128)
    pswap = np.zeros((128, 128), dtype=np.float32)
    pswap[r_idx ^ 32, r_idx] = 1.0
    pswap = pswap.astype(BF16)

    def part_major(a, p=128):  # [R, cols] with R = n*p -> [p, n*cols]
        R, cols = a.shape
        n = R // p
        return np.ascontiguousarray(
            a.reshape(n, p, cols).transpose(1, 0, 2).reshape(p, n * cols)
        )

    in_maps = []
    for core in range(8):
        b, hg = divmod(core, 4)
        xT = x[b].T.astype(BF16)                                       # [C, T]
        # [C, T] -> [128, jc*(8*512)]: (c p), (jc t') -> p, jc, c, t'
        xt_pm = np.ascontiguousarray(
            xT.reshape(8, 128, NCHUNK, 512)
            .transpose(1, 2, 0, 3)
            .reshape(128, NCHUNK * 8 * 512)
        )
        heads = []
        for h in range(hg * NQ, hg * NQ + NQ):
            heads.append(W_qkv[:, h * HD : (h + 1) * HD][:, perm])     # [C, 64]
        kcol = W_qkv[:, NH * HD + hg * HD : NH * HD + (hg + 1) * HD][:, perm]
        blocks = [
            np.concatenate([heads[0], heads[1]], axis=1),              # q0
            np.concatenate([heads[2], heads[3]], axis=1),              # q1
            np.concatenate([kcol, kcol], axis=1),                      # k dup
        ]
        wqT = np.concatenate(blocks, axis=1).astype(BF16)              # [C, 384]
        wv = W_qkv[
            :, (NH + NKV) * HD + hg * HD : (NH + NKV) * HD + (hg + 1) * HD
        ].astype(BF16)                                                 # [C, 64]
        wo = W_proj[hg * NQ * HD : (hg + 1) * NQ * HD, :].astype(BF16)
        in_maps.append(
            {
                "xT": xt_pm,
                "wqT": part_major(wqT),
                "wv": part_major(wv),
                "pswap": pswap,
                "cs2": cs2,
                "wo": part_major(wo),
            }
        )
    return in_maps


def _run(in_maps):
    from concourse.bass_utils import run_bass_kernel_spmd

    if "nc" not in _CACHE:
        _CACHE["nc"] = _build()
    return run_bass_kernel_spmd(_CACHE["nc"], in_maps, core_ids=list(range(8)))


def _get_nc(reps=1):
    key = "nc" if reps == 1 else f"nc{reps}"
    if key not in _CACHE:
        _CACHE[key] = _build(reps)
    return _CACHE[key]


def kernel(x, W_qkv, W_proj):
    x = np.asarray(x, dtype=np.float32)
    W_qkv = np.asarray(W_qkv, dtype=np.float32)
    W_proj = np.asarray(W_proj, dtype=np.float32)
    res = _run(_host_inputs(x, W_qkv, W_proj))
    out = np.zeros((B, T, C), dtype=np.float32)
    for core in range(8):
        b = core // 4
        o = res.results[core]["out"].astype(np.float32)  # [128, TT*C] p-major
        out[b] += o.reshape(128, TT, C).transpose(1, 0, 2).reshape(T, C)
    return out


# revision 24
# speedup vs baseline: 1.0348x; 1.0348x over previous
"""Causal GQA self-attention (B=2, T=2048, C=1024, 16 q-heads / 4 kv-heads,
RoPE, causal softmax, output projection) on 8 Trainium2 NeuronCores.

Sharding: core c = b*4 + hg handles batch b (2-way data parallel) and
head-group hg (4-way tensor parallel: its 4 q-heads + their shared kv head).
W_qkv is column-sharded, W_proj row-sharded; each core emits a partial
projection [2048, 1024] and the host sums the 4 partials per batch.

Device pipeline per core (bf16 matmul inputs, fp32 PSUM accumulate):
  1. q^T and k^T computed DIRECTLY in [head-dim, token] layout:
     qT_raw = W_stack^T @ x^T (host pre-transposes; two 64-dim q heads per
     128-wide block; the k block is [W_k | W_k] so k^T lands duplicated on
     both partition halves for the two-head score trick). RoPE pair-swap
     comes from ONE cheap 128x128 permutation matmul (pswap) per stack:
     qT_rot = qT_raw*cos + (P @ qT_raw)*ssin, with the rotation sign baked
     into the host ssin table. No DMA transposes for q/k.
  2. v^T = W_v^T @ x^T (8 wide matmuls), then 4 small DMA-xbar transposes
     per chunk into natural [token, dim] layout + ones column (softmax
     denominator row via the [1|v] stationary trick).
  3. flash-style per 512-wide q chunk: for each 128-token k tile, the two
     heads' scores land in ONE 2-bank PSUM tile [128, 2, 512]; a single exp
     on ScalarE covers both heads (1/sqrt(64) folded into the activation
     scale); causal masking via one 3D affine_select on diagonal blocks;
     y^T[65, 2, q] += [1|v]^T @ P^T with a single merged matmul.
  4. y^T row 0 is the softmax denominator: reciprocal_approx + GPSIMD
     partition_broadcast + one fused scale-evacuate multiply
  5. out = yT.T @ W_proj_shard, stored partition-major with one DMA per
     chunk (128 descriptors each).
"""

import sys

if "/opt/trn_rl_repo" not in sys.path:
    sys.path.insert(0, "/opt/trn_rl_repo")

import numpy as np
import ml_dtypes

B, T, C = 2, 2048, 1024
NH, NKV, HD = 16, 4, 64
THETA = 10000.0
NQ = NH // NKV          # q heads per core = 4
TT = T // 128           # 16 token tiles
NCHUNK = T // 512       # 4 q-chunks
BF16 = ml_dtypes.bfloat16

_CACHE = {}


def _null_ctx():
    from contextlib import nullcontext

    return nullcontext()


def _build(reps=1, merged_pv=False):
    """Build the SPMD Bass program (identical on all 8 cores).

    reps>1 wraps the whole body in a hardware loop (constant NEFF size) —
    used only by hw_time.py to measure per-iteration device time.
    """
    import concourse.mybir as mybir
    import concourse.tile as tile
    from concourse import bacc
    from concourse.bass import ts
    from contextlib import ExitStack

    dt = mybir.dt
    AF = mybir.ActivationFunctionType

    nc = bacc.Bacc("TRN2", target_bir_lowering=False, debug=False, num_devices=8)

    # host pre-shuffled, partition-major inputs (contiguous per partition)
    xt_d = nc.declare_dram_parameter("xT", [128, 8 * T], dt.bfloat16, isOutput=False)
    wq_d = nc.declare_dram_parameter("wqT", [128, 8 * 384], dt.bfloat16, isOutput=False)
    wv_d = nc.declare_dram_parameter("wv", [128, 8 * 64], dt.bfloat16, isOutput=False)
    ps_d = nc.declare_dram_parameter("pswap", [128, 128], dt.bfloat16, isOutput=False)
    cs_d = nc.declare_dram_parameter("cs2", [128, 2 * T], dt.bfloat16, isOutput=False)
    wo_d = nc.declare_dram_parameter("wo", [128, 2 * C], dt.bfloat16, isOutput=False)
    # partition-major output: row p holds (p, tt, :) — 128 descriptors/DMA
    out_d = nc.declare_dram_parameter("out", [128, TT * C], dt.bfloat16, isOutput=True)

    with tile.TileContext(nc) as tc:
     with (tc.For_i(0, reps) if reps > 1 else _null_ctx()):
      with ExitStack() as ctx:
        persist = ctx.enter_context(tc.tile_pool(name="persist", bufs=1))
        cmb_tmp = ctx.enter_context(tc.tile_pool(name="cmb_tmp", bufs=3))
        raw_pool = ctx.enter_context(tc.tile_pool(name="raw", bufs=3))
        p_pool = ctx.enter_context(tc.tile_pool(name="p_pool", bufs=8))
        po_pool = ctx.enter_context(tc.tile_pool(name="po", bufs=2))
        ysb_pool = ctx.enter_context(tc.tile_pool(name="ysb", bufs=3))
        bc_pool = ctx.enter_context(tc.tile_pool(name="bc", bufs=2))
        yst_pool = ctx.enter_context(tc.tile_pool(name="yst", bufs=6))
        s_ps_pool = ctx.enter_context(
            tc.tile_pool(name="s_ps", bufs=2, space="PSUM")
        )
        qk_ps_pool = ctx.enter_context(
            tc.tile_pool(name="qk_ps", bufs=2, space="PSUM")
        )
        y_ps_pool = ctx.enter_context(
            tc.tile_pool(name="y_ps", bufs=1, space="PSUM")
        )

        # ---- persistent SBUF; DMA order tuned so the first qkv matmul
        # group (xt chunk 0 + wq + pswap) lands before cs/wv/wo ----
        wq_sb = persist.tile([128, 8, 384], dt.bfloat16)
        ps_sb = persist.tile([128, 128], dt.bfloat16)
        wv_sb = persist.tile([128, 8, 64], dt.bfloat16)
        cs_sb = persist.tile([128, 2, T], dt.bfloat16)
        wo_sb = persist.tile([128, 2, C], dt.bfloat16)
        xt_sb = [
            persist.tile([128, 8, 512], dt.bfloat16, name=f"xtc{jc}")
            for jc in range(NCHUNK)
        ]
        nc.sync.dma_start(
            wq_sb[:, 0:2, :],
            wq_d.ap()[:, 0 : 2 * 384].rearrange("p (c n) -> p c n", c=2),
        )
        nc.sync.dma_start(ps_sb[:], ps_d.ap())
        nc.sync.dma_start(
            xt_sb[0][:, 0:2, :],
            xt_d.ap()[:, 0:1024].rearrange("p (c t) -> p c t", c=2),
        )
        nc.sync.dma_start(
            wq_sb[:, 2:8, :],
            wq_d.ap()[:, 2 * 384 :].rearrange("p (c n) -> p c n", c=6),
        )
        nc.sync.dma_start(
            xt_sb[0][:, 2:8, :],
            xt_d.ap()[:, 1024 : 8 * 512].rearrange("p (c t) -> p c t", c=6),
        )
        nc.sync.dma_start(
            cs_sb[:], cs_d.ap().rearrange("p (v t) -> p v t", v=2)
        )
        nc.sync.dma_start(
            wv_sb[:], wv_d.ap().rearrange("p (c n) -> p c n", c=8)
        )
        for jc in range(1, NCHUNK):
            nc.sync.dma_start(
                xt_sb[jc][:],
                xt_d.ap()[:, ts(jc, 8 * 512)].rearrange("p (c t) -> p c t", c=8),
            )
        nc.sync.dma_start(
            wo_sb[:], wo_d.ap().rearrange("p (c n) -> p c n", c=2)
        )

        qt_sb = [[None] * NCHUNK for _ in range(2)]   # [hp][chunk] [128,512]
        kt_sb = []                                    # per chunk [128,512] (dup)
        va_sb = []                                    # per chunk [128,4,65]
        ynt = [[None] * NCHUNK for _ in range(2)]     # [dimtile][chunk] [128,512]
        for d in range(2):
            for j in range(NCHUNK):
                qt_sb[d][j] = persist.tile([128, 512], dt.bfloat16, name=f"qt{d}_{j}")
                ynt[d][j] = persist.tile([128, 512], dt.bfloat16, name=f"ynt{d}_{j}")

        # ---- phase 1: qT/kT via matmul + perm-matmul rope; vT + transpose ----
        for jc in range(NCHUNK):
            kt = persist.tile([128, 512], dt.bfloat16, name=f"kt{jc}")
            kt_sb.append(kt)
            va = persist.tile([128, 4, 65], dt.bfloat16, name=f"va{jc}")
            va_sb.append(va)
            cos_sl = cs_sb[:, 0, ts(jc, 512)]
            sin_sl = cs_sb[:, 1, ts(jc, 512)]
            # block order in wqT: q0 | q1 | k
            for bA, dst in ((2, kt), (0, qt_sb[0][jc]), (1, qt_sb[1][jc])):
                psA = qk_ps_pool.tile([128, 512], dt.float32, tag="qk", name="psA")
                for c in range(8):
                    nc.tensor.matmul(
                        psA[:],
                        lhsT=wq_sb[:, c, ts(bA, 128)],
                        rhs=xt_sb[jc][:, c, :],
                        start=(c == 0),
                        stop=(c == 7),
                    )
                raw = raw_pool.tile([128, 512], dt.bfloat16, tag="raw")
                nc.vector.tensor_copy(raw[:], psA[:])
                psB = qk_ps_pool.tile([128, 512], dt.float32, tag="qk", name="psB")
                nc.tensor.matmul(
                    psB[:], lhsT=ps_sb[:], rhs=raw[:], start=True, stop=True
                )
                t1 = cmb_tmp.tile([128, 512], dt.bfloat16, tag="t1")
                t2 = cmb_tmp.tile([128, 512], dt.bfloat16, tag="t2")
                nc.vector.tensor_mul(t1[:], raw[:], cos_sl)
                nc.vector.tensor_mul(t2[:], psB[:], sin_sl)
                nc.vector.tensor_add(dst[:], t1[:], t2[:])
            # vT then 4 xbar transposes into natural layout (off crit path)
            psV = qk_ps_pool.tile([64, 512], dt.float32, tag="qk", name="psV")
            for c in range(8):
                nc.tensor.matmul(
                    psV[:],
                    lhsT=wv_sb[:, c, :],
                    rhs=xt_sb[jc][:, c, :],
                    start=(c == 0),
                    stop=(c == 7),
                )
            vt = raw_pool.tile([64, 512], dt.bfloat16, tag="vt")
            nc.vector.tensor_copy(vt[:], psV[:])
            vn = raw_pool.tile([128, 4, 64], dt.bfloat16, tag="vn")
            for t4 in range(4):
                nc.sync.dma_start_transpose(
                    vn[:, t4, :], vt[:, ts(t4, 128)]
                )
            nc.vector.tensor_copy(va[:, :, 1:65], vn[:])
            nc.gpsimd.memset(va[:, :, 0:1], 1.0)

        # ---- phase 3+4: attention + projection per 512-wide q chunk ----
        # y^T[65, 2, q] = [1|v]^T @ P^T over k tiles; row 0 = denominator.
        # Both heads of a pair share one 2-bank score tile and one exp.
        for j in range(NCHUNK):
            for hp in range(2):
                y_ps = y_ps_pool.tile(
                    [65, 2, 512], dt.float32, tag="y", name="y_ps"
                )
                for i in range(4 * j + 4):  # k tiles
                    ic, i4 = divmod(i, 4)
                    off = max(0, 128 * i - 512 * j)  # causal: valid q >= 128*i
                    s2 = s_ps_pool.tile(
                        [128, 2, 512], dt.float32, tag="s", name="s2"
                    )
                    for u in range(2):  # head 2hp+u, kT copy at partitions 64u
                        nc.tensor.matmul(
                            s2[:, u, off:512],
                            lhsT=kt_sb[ic][ts(u, 64), ts(i4, 128)],
                            rhs=qt_sb[hp][j][ts(u, 64), off:512],
                            start=True,
                            stop=True,
                        )
                    p_t = p_pool.tile([128, 2, 512], dt.bfloat16, name="p_t")
                    nc.scalar.activation(
                        p_t[:, :, off:512], s2[:, :, off:512], AF.Exp, scale=0.125
                    )
                    if 128 * i >= 512 * j:  # diagonal block: causal mask
                        # keep where q_local - k_local >= 0, else 0 (both heads)
                        nc.gpsimd.affine_select(
                            p_t[:, :, off : off + 128],
                            p_t[:, :, off : off + 128],
                            pattern=[[0, 2], [1, 128]],
                            compare_op=mybir.AluOpType.is_ge,
                            fill=0.0,
                            base=0,
                            channel_multiplier=-1,
                        )
                    if merged_pv:
                        nc.tensor.matmul(
                            y_ps[:, :, off:512],
                            lhsT=va_sb[ic][:, i4, 0:65],
                            rhs=p_t[:, :, off:512],
                            start=(i == 0),
                            stop=(i == 4 * j + 3),
                        )
                    else:
                        for u in range(2):
                            nc.tensor.matmul(
                                y_ps[:, u, off:512],
                                lhsT=va_sb[ic][:, i4, 0:65],
                                rhs=p_t[:, u, off:512],
                                start=(i == 0),
                                stop=(i == 4 * j + 3),
                            )
                # evacuate unnormalized y + den quickly to free the psum banks
                y_sb = ysb_pool.tile([65, 2, 512], dt.float32)
                nc.vector.tensor_copy(y_sb[:], y_ps[:])
                # den row -> reciprocal -> broadcast -> scale
                nc.vector.reciprocal_approx_fast(y_sb[0:1, :, :], y_sb[0:1, :, :])
                bc = bc_pool.tile([65, 2, 512], dt.float32)
                nc.gpsimd.partition_broadcast(bc[:], y_sb[0:1, :, :], channels=65)
                for u in range(2):
                    yst = yst_pool.tile([65, 512], dt.bfloat16)
                    nc.vector.tensor_mul(yst[:, :], y_sb[:, u, :], bc[:, u, :])
                    if j == NCHUNK - 1 and hp == 1:
                        nc.scalar.dma_start(
                            ynt[hp][j][ts(u, 64), :], yst[1:65, :]
                        )
                    else:
                        nc.gpsimd.dma_start(
                            ynt[hp][j][ts(u, 64), :], yst[1:65, :]
                        )
            # projection for this chunk's 4 token tiles, one store per chunk
            po = po_pool.tile([128, 4, C], dt.bfloat16)
            for t4 in range(4):
                for nn2 in range(2):
                    ps = qk_ps_pool.tile(
                        [128, 512], dt.float32, tag="qk", name="pr_ps"
                    )
                    for dtile in range(2):
                        nc.tensor.matmul(
                            ps[:],
                            lhsT=ynt[dtile][j][:, ts(t4, 128)],
                            rhs=wo_sb[:, dtile, ts(nn2, 512)],
                            start=(dtile == 0),
                            stop=(dtile == 1),
                        )
                    nc.vector.tensor_copy(po[:, t4, ts(nn2, 512)], ps[:])
            nc.sync.dma_start(
                out_d.ap()[:, ts(j, 4 * C)],
                po[:].rearrange("p t n -> p (t n)"),
            )

    nc.finalize()
    return nc


def _host_inputs(x, W_qkv, W_proj):
    """Per-core input maps (host-side sharding + partition-major layout)."""
    perm = np.concatenate([np.arange(0, HD, 2), np.arange(1, HD, 2)])  # even|odd
    inv = 1.0 / THETA ** (np.arange(0, HD, 2, dtype=np.float64) / HD)  # [32]
    ang = np.arange(T, dtype=np.float64)[:, None] * inv[None, :]       # [T, 32]
    # cos rows p: cos(ang[t, p mod 32]); ssin rows: -sin on the x1 half
    # (p mod 64 < 32), +sin on the x2 half
    cosT = np.cos(ang).T                                               # [32, T]
    sinT = np.sin(ang).T
    cos128 = np.tile(cosT, (4, 1))                                     # [128, T]
    ssin64 = np.concatenate([-sinT, sinT], axis=0)                     # [64, T]
    ssin128 = np.tile(ssin64, (2, 1))
    cs2 = np.concatenate([cos128, ssin128], axis=1).astype(BF16)       # [128,2T]
    # pair-swap permutation: pswap[k, r] = 1 iff r == k xor 32
    r_idx = np.arange(128)
    pswap = np.zeros((128, 128), dtype=np.float32)
    pswap[r_idx ^ 32, r_idx] = 1.0
    pswap = pswap.astype(BF16)

    def part_major(a, p=128):  # [R, cols] with R = n*p -> [p, n*cols]
        R, cols = a.shape
        n = R // p
        return np.ascontiguousarray(
            a.reshape(n, p, cols).transpose(1, 0, 2).reshape(p, n * cols)
        )

    in_maps = []
    for core in range(8):
        b, hg = divmod(core, 4)
        xT = x[b].T.astype(BF16)                                       # [C, T]
        # [C, T] -> [128, jc*(8*512)]: (c p), (jc t') -> p, jc, c, t'
        xt_pm = np.ascontiguousarray(
            xT.reshape(8, 128, NCHUNK, 512)
            .transpose(1, 2, 0, 3)
            .reshape(128, NCHUNK * 8 * 512)
        )
        heads = []
        for h in range(hg * NQ, hg * NQ + NQ):
            heads.append(W_qkv[:, h * HD : (h + 1) * HD][:, perm])     # [C, 64]
        kcol = W_qkv[:, NH * HD + hg * HD : NH * HD + (hg + 1) * HD][:, perm]
        blocks = [
            np.concatenate([heads[0], heads[1]], axis=1),              # q0
            np.concatenate([heads[2], heads[3]], axis=1),              # q1
            np.concatenate([kcol, kcol], axis=1),                      # k dup
        ]
        wqT = np.concatenate(blocks, axis=1).astype(BF16)              # [C, 384]
        wv = W_qkv[
            :, (NH + NKV) * HD + hg * HD : (NH + NKV) * HD + (hg + 1) * HD
        ].astype(BF16)                                                 # [C, 64]
        wo = W_proj[hg * NQ * HD : (hg + 1) * NQ * HD, :].astype(BF16)
        in_maps.append(
            {
                "xT": xt_pm,
                "wqT": part_major(wqT),
                "wv": part_major(wv),
                "pswap": pswap,
                "cs2": cs2,
                "wo": part_major(wo),
            }
        )
    return in_maps


def _run(in_maps):
    from concourse.bass_utils import run_bass_kernel_spmd

    if "nc" not in _CACHE:
        _CACHE["nc"] = _build()
    return run_bass_kernel_spmd(_CACHE["nc"], in_maps, core_ids=list(range(8)))


def _get_nc(reps=1):
    key = "nc" if reps == 1 else f"nc{reps}"
    if key not in _CACHE:
        _CACHE[key] = _build(reps)
    return _CACHE[key]


def kernel(x, W_qkv, W_proj):
    x = np.asarray(x, dtype=np.float32)
    W_qkv = np.asarray(W_qkv, dtype=np.float32)
    W_proj = np.asarray(W_proj, dtype=np.float32)
    res = _run(_host_inputs(x, W_qkv, W_proj))
    out = np.zeros((B, T, C), dtype=np.float32)
    for core in range(8):
        b = core // 4
        o = res.results[core]["out"].astype(np.float32)  # [128, TT*C] p-major
        out[b] += o.reshape(128, TT, C).transpose(1, 0, 2).reshape(T, C)
    return out
